# revision 2
# baseline (speedup 1.0000x reference)
"""Trainium2 Bass kernel for nn_DGLJTNNEncoder (junction-tree GRU encoder).

Forest of B=512 perfect binary trees (depth 6, H=256, V=780). The reference
runs an edge-GRU message passing over up+down BFS levels, then a gather at
every node, returning h[roots]. Only the upward pass reaches the roots, so
the kernel computes just the upward levels, sharded 64 trees per NeuronCore
across 8 cores.

Layout: activations are column-major [H (2 partition tiles of 128), L]. Edges
of each level are ordered so that the two child edges of level-d edge i sit
at positions i and i+L/2 of level d+1 -> predecessor sums are stride-1 adds.
Leaf-edge messages are a pure vocab lookup: m_leaf = sigmoid(emb@Wz_top+bz)
* tanh(emb@Wh_top+bh) gathered from a 780-entry table via gpsimd ap_gather.
"""

import numpy as np

B, D, H, V = 512, 6, 256, 780
N_TREE = 127
NCORES = 8
T = B // NCORES                                  # 64 trees per core
LW = {d: T * (1 << d) for d in range(D + 1)}     # level width (edges) per core
NL = sum(LW[d] for d in range(D))                # 4032 non-leaf nodes (depths 0..5)
LF = LW[D]                                       # 4096 leaves


def _orders():
    ords = [np.stack([np.arange(T), np.zeros(T, np.int64)], 1)]
    for _ in range(D):
        t, c = ords[-1][:, 0], ords[-1][:, 1]
        ords.append(np.concatenate(
            [np.stack([t, 2 * c + 1], 1), np.stack([t, 2 * c + 2], 1)], 0))
    return ords


_ORDS = _orders()
_NL_NODE = np.concatenate([_ORDS[d] for d in (5, 4, 3, 2, 1, 0)], 0)  # [4032,(t,c)]
_LF_NODE = _ORDS[6]                                                   # [4096,(t,c)]
_OFF = {}
_o = 0
for _d in (5, 4, 3, 2, 1, 0):
    _OFF[_d] = _o
    _o += LW[_d]


def _wrap(idx):
    """[n] ints -> [128, n//16] int16 ap_gather wrap layout (replicated per Q7 core)."""
    n = idx.shape[0]
    blk = idx.reshape(n // 16, 16).T.astype(np.int16)
    return np.ascontiguousarray(np.tile(blk, (8, 1)))


_NC_CACHE = None


def _build(phase=4):
    import concourse.mybir as mybir
    import concourse.tile as tile
    from concourse import bacc
    from contextlib import ExitStack

    f32, f32r, i16 = mybir.dt.float32, mybir.dt.float32r, mybir.dt.int16
    AF, OP = mybir.ActivationFunctionType, mybir.AluOpType

    nc = bacc.Bacc("TRN2", dynamic_dma_scratch_size=4096)
    emb = nc.dram_tensor("emb", [V, H], f32, kind="ExternalInput")
    Wz = nc.dram_tensor("Wz", [2 * H, H], f32, kind="ExternalInput")
    Wh = nc.dram_tensor("Wh", [2 * H, H], f32, kind="ExternalInput")
    Wr = nc.dram_tensor("Wr", [H, H], f32, kind="ExternalInput")
    Ur = nc.dram_tensor("Ur", [H, H], f32, kind="ExternalInput")
    Wg = nc.dram_tensor("Wg", [2 * H, H], f32, kind="ExternalInput")
    bz = nc.dram_tensor("bz", [H], f32, kind="ExternalInput")
    bh = nc.dram_tensor("bh", [H], f32, kind="ExternalInput")
    bur = nc.dram_tensor("bur", [H], f32, kind="ExternalInput")
    bg = nc.dram_tensor("bg", [H], f32, kind="ExternalInput")
    widn = nc.dram_tensor("widn", [128, NL // 16], i16, kind="ExternalInput")
    widl = nc.dram_tensor("widl", [128, LF // 16], i16, kind="ExternalInput")
    ident = nc.dram_tensor("ident", [128, 128], f32, kind="ExternalInput")
    out = nc.dram_tensor("out", [H, T], f32, kind="ExternalOutput")
    dbg = (nc.dram_tensor("dbg", [H, 8192], f32, kind="ExternalOutput")
           if phase < 4 else None)

    with ExitStack() as ctx:
        tc = ctx.enter_context(tile.TileContext(nc))
        SB = ctx.enter_context(tc.tile_pool(name="sb", bufs=1))
        TP = ctx.enter_context(tc.tile_pool(name="tp", bufs=3))
        PS = ctx.enter_context(tc.tile_pool(name="ps", bufs=6, space="PSUM"))

        # ---- weights/biases/indices into SBUF (one DMA per tensor) ----
        def wload(name, src, kt):
            t = SB.tile([128, kt * H], f32r, tag=name, name=name)
            nc.sync.dma_start(
                t[:].rearrange("p (k h) -> p k h", h=H),
                src[:].rearrange("(k p) h -> p k h", p=128).bitcast(f32r))
            return [t[:, H * k:H * (k + 1)] for k in range(kt)]

        Wz_sb = wload("Wz", Wz, 4)
        Wh_sb = wload("Wh", Wh, 4)
        Wr_sb = wload("Wr", Wr, 2)
        Ur_sb = wload("Ur", Ur, 2)
        Wg_sb = wload("Wg", Wg, 4)

        def bload(name, src):
            t = SB.tile([128, 2], f32, tag=name)
            nc.sync.dma_start(t[:], src[:].rearrange("(m p) -> p m", p=128))
            return t

        bz_sb, bh_sb, bur_sb, bg_sb = (bload(n, s) for n, s in
                                       (("bz", bz), ("bh", bh), ("bur", bur), ("bg", bg)))
        id_sb = SB.tile([128, 128], f32, tag="ident", name="ident")
        nc.sync.dma_start(id_sb[:], ident[:])
        widn_sb = SB.tile([128, NL // 16], i16, tag="widn", name="widn")
        nc.sync.dma_start(widn_sb[:], widn[:])
        widl_sb = SB.tile([128, LF // 16], i16, tag="widl", name="widl")
        nc.sync.dma_start(widl_sb[:], widl[:])

        # ---- emb -> embT (column layout, f32 for ap_gather) ----
        embT = [SB.tile([128, V], f32, tag=f"embT{k}", name=f"embT{k}") for k in range(2)]
        embTr = [SB.tile([128, V], f32r, tag=f"embTr{k}", name=f"embTr{k}") for k in range(2)]
        est = SB.tile([128, 6 * H], f32, tag="est", name="est")
        nc.sync.dma_start(est[:].rearrange("p (t h) -> p t h", h=H),
                          emb[:768, :].rearrange("(t p) h -> p t h", p=128))
        etail = SB.tile([128, H], f32, tag="etail", name="etail")
        nc.sync.dma_start(etail[:12, :], emb[768:, :])
        for ti in range(7):
            rows = 128 if ti < 6 else V - 768
            for k in range(2):
                src_ap = (est[:, H * ti + 128 * k:H * ti + 128 * (k + 1)] if ti < 6
                          else etail[:rows, 128 * k:128 * (k + 1)])
                pt = PS.tile([128, 512], f32, tag="ps", name="ps")
                nc.tensor.transpose(pt[:, :rows], src_ap, id_sb[:rows, :rows])
                nc.scalar.copy(embT[k][:, 128 * ti:128 * ti + rows], pt[:, :rows])
                nc.vector.tensor_copy(embTr[k][:, 128 * ti:128 * ti + rows], pt[:, :rows])

        # ---- xA gather, leaf-critical chunk first (cols 0..2048 = X5/x_dst) ----
        xA = [SB.tile([128, NL], f32r, tag=f"xA{k}", name=f"xA{k}") for k in range(2)]
        stage = SB.tile([128, 4096], f32, tag="stage", name="stage")

        def gather_rc(dst, tab, idx_t, c0, cw, soff):
            nc.gpsimd.ap_gather(out_ap=stage[:, soff:soff + cw], in_ap=tab[:],
                                idxs_ap=idx_t[:, c0 // 16:(c0 + cw) // 16],
                                channels=128, num_elems=V, d=1, num_idxs=cw)
            nc.sync.dma_start(dst[:, c0:c0 + cw],
                              stage[:, soff:soff + cw].bitcast(f32r))

        for k in range(2):
            gather_rc(xA[k], embT[k], widn_sb, 0, 2048, 2048 * k)

        # ---- leaf message table: Tm = sigmoid(Wz_top^T x + bz) * tanh(Wh_top^T x + bh) ----
        Tm = [SB.tile([128, V], f32, tag=f"Tm{k}", name=f"Tm{k}") for k in range(2)]
        for h2 in range(2):
            for c0, cw in ((0, 512), (512, V - 512)):
                pz = PS.tile([128, 512], f32, tag="ps", name="ps")
                ph = PS.tile([128, 512], f32, tag="ps", name="ps")
                for k in range(2):
                    nc.tensor.matmul(pz[:, :cw], Wz_sb[k][:, 128 * h2:128 * (h2 + 1)],
                                     embTr[k][:, c0:c0 + cw], start=(k == 0), stop=(k == 1))
                for k in range(2):
                    nc.tensor.matmul(ph[:, :cw], Wh_sb[k][:, 128 * h2:128 * (h2 + 1)],
                                     embTr[k][:, c0:c0 + cw], start=(k == 0), stop=(k == 1))
                zt = TP.tile([128, 512], f32, tag="ew", name="ztab", bufs=4)
                tt = TP.tile([128, 512], f32, tag="ew", name="ttab", bufs=4)
                nc.scalar.activation(zt[:, :cw], pz[:, :cw], AF.Sigmoid, bias=bz_sb[:, h2:h2 + 1])
                nc.scalar.activation(tt[:, :cw], ph[:, :cw], AF.Tanh, bias=bh_sb[:, h2:h2 + 1])
                nc.vector.tensor_tensor(Tm[h2][:, c0:c0 + cw], zt[:, :cw], tt[:, :cw], op=OP.mult)

        # ---- mA gather: mA = Tm[:, wid[leaf]] (f32 -> recast f32r) ----
        mA = [SB.tile([128, LF], f32r, tag=f"mA{k}", name=f"mA{k}") for k in range(2)]
        for k in range(2):
            for ci, c0 in enumerate((0, 2048)):
                gather_rc(mA[k], Tm[k], widl_sb, c0, 2048, 2048 * ((k + ci) % 2))
        # late xA chunk (cols 2048..4032, first needed at level 4)
        for k in range(2):
            gather_rc(xA[k], embT[k], widn_sb, 2048, 1984, 2048 * k)

        if phase == 1:
            for k in range(2):
                nc.sync.dma_start(dbg[128 * k:128 * (k + 1), 0:NL], xA[k][:].bitcast(f32))
                nc.sync.dma_start(dbg[128 * k:128 * (k + 1), 4096:4096 + LF], mA[k][:].bitcast(f32))
        # ---- leaf level (d=6): m = mA; r = sig(Wr^T x_dst + Ur^T m + bur); rm = r*m ----
        # s5 = pairsum(mA), arm5 = pairsum(rm6) -- slab pairs (c0, c0+L/2)
        L5 = LW[5]
        s_nxt = [SB.tile([128, L5], f32r, tag=f"s5_{k}", name=f"s5_{k}") for k in range(2)]
        arm_nxt = [SB.tile([128, L5], f32r, tag=f"a5_{k}", name=f"a5_{k}") for k in range(2)]
        for k in range(2):
            nc.vector.tensor_tensor(s_nxt[k][:], mA[k][:, :L5], mA[k][:, L5:], op=OP.add)
        for c0 in range(0, L5, 512):
            rm_halves = []
            for half in range(2):
                cc = c0 + half * L5
                rms = []
                for h2 in range(2):
                    pr = PS.tile([128, 512], f32, tag="ps", name="ps")
                    args = [(Wr_sb[0], xA[0][:, c0:c0 + 512]),
                            (Wr_sb[1], xA[1][:, c0:c0 + 512]),
                            (Ur_sb[0], mA[0][:, cc:cc + 512]),
                            (Ur_sb[1], mA[1][:, cc:cc + 512])]
                    for i, (w, rhs) in enumerate(args):
                        nc.tensor.matmul(pr[:], w[:, 128 * h2:128 * (h2 + 1)], rhs,
                                         start=(i == 0), stop=(i == 3))
                    r_t = TP.tile([128, 512], f32, tag="rr", name="r_t", bufs=3)
                    nc.scalar.activation(r_t[:], pr[:], AF.Sigmoid, bias=bur_sb[:, h2:h2 + 1])
                    rm_t = TP.tile([128, 512], f32, tag=f"rm{h2}", name="rm_t", bufs=2)
                    nc.gpsimd.tensor_tensor(rm_t[:], r_t[:], mA[h2][:, cc:cc + 512], op=OP.mult)
                    rms.append(rm_t)
                rm_halves.append(rms)
            for h2 in range(2):
                nc.vector.tensor_tensor(arm_nxt[h2][:, c0:c0 + 512],
                                        rm_halves[0][h2][:], rm_halves[1][h2][:], op=OP.add)

        if phase == 2:
            for k in range(2):
                nc.sync.dma_start(dbg[128 * k:128 * (k + 1), 0:LW[5]], s_nxt[k][:].bitcast(f32))
                nc.sync.dma_start(dbg[128 * k:128 * (k + 1), 4096:4096 + LW[5]], arm_nxt[k][:].bitcast(f32))
        # ---- levels d = 5..1 ----
        levels = {3: (5,), 4: (5, 4, 3, 2, 1)}.get(phase, ())
        for d in levels:
            L = LW[d]
            s_cur, arm_cur = s_nxt, arm_nxt
            X = [xA[k][:, _OFF[d]:_OFF[d] + L] for k in range(2)]
            Xp = [xA[k][:, _OFF[d - 1]:_OFF[d - 1] + LW[d - 1]] for k in range(2)]
            S = max(64, min(512, L // 2))
            s_nxt = [SB.tile([128, L // 2], f32r, tag=f"s{d - 1}_{k}", name=f"s{d - 1}_{k}") for k in range(2)]
            if d >= 2:
                arm_nxt = [SB.tile([128, L // 2], f32r, tag=f"a{d - 1}_{k}", name=f"a{d - 1}_{k}") for k in range(2)]
            else:
                arm_nxt = None
            if L <= 512:
                # single full-level slab: N=L matmuls (avoids f32r N<256 4x penalty)
                m_t = [TP.tile([128, 512], f32r, tag="mh", name="m_t", bufs=4)[:, :L]
                       for k in range(2)]
                rm_t = [TP.tile([128, 512], f32, tag=f"rm{k}", name="rm_t", bufs=2)[:, :L]
                        for k in range(2)]
                for h2 in range(2):
                    pz = PS.tile([128, 512], f32, tag="ps", name="ps")
                    ph = PS.tile([128, 512], f32, tag="ps", name="ps")
                    argz = [(Wz_sb[0], X[0]), (Wz_sb[1], X[1]),
                            (Wz_sb[2], s_cur[0][:, :L]), (Wz_sb[3], s_cur[1][:, :L])]
                    for i, (w, rhs) in enumerate(argz):
                        nc.tensor.matmul(pz[:, :L], w[:, 128 * h2:128 * (h2 + 1)], rhs,
                                         start=(i == 0), stop=(i == 3))
                    argh = [(Wh_sb[0], X[0]), (Wh_sb[1], X[1]),
                            (Wh_sb[2], arm_cur[0][:, :L]), (Wh_sb[3], arm_cur[1][:, :L])]
                    for i, (w, rhs) in enumerate(argh):
                        nc.tensor.matmul(ph[:, :L], w[:, 128 * h2:128 * (h2 + 1)], rhs,
                                         start=(i == 0), stop=(i == 3))
                    z_t = TP.tile([128, 512], f32, tag="ew", name="z_t", bufs=4)[:, :L]
                    t_t = TP.tile([128, 512], f32, tag="ew", name="t_t", bufs=4)[:, :L]
                    nc.scalar.activation(z_t[:], pz[:, :L], AF.Sigmoid, bias=bz_sb[:, h2:h2 + 1])
                    nc.scalar.activation(t_t[:], ph[:, :L], AF.Tanh, bias=bh_sb[:, h2:h2 + 1])
                    dd = TP.tile([128, 512], f32, tag="ew", name="dd", bufs=4)[:, :L]
                    nc.vector.tensor_tensor(dd[:], t_t[:], s_cur[h2][:, :L], op=OP.subtract)
                    ee = TP.tile([128, 512], f32, tag="ew", name="ee", bufs=4)[:, :L]
                    nc.vector.tensor_tensor(ee[:], z_t[:], dd[:], op=OP.mult)
                    nc.vector.tensor_tensor(m_t[h2][:], ee[:], s_cur[h2][:, :L], op=OP.add)
                if d >= 2:
                    for h2 in range(2):
                        pr = PS.tile([128, 512], f32, tag="ps", name="ps")
                        for half in range(2):
                            seg = slice(half * (L // 2), (half + 1) * (L // 2))
                            argr = [(Wr_sb[0], Xp[0]), (Wr_sb[1], Xp[1]),
                                    (Ur_sb[0], m_t[0][:, seg]), (Ur_sb[1], m_t[1][:, seg])]
                            for i, (w, rhs) in enumerate(argr):
                                nc.tensor.matmul(pr[:, seg], w[:, 128 * h2:128 * (h2 + 1)],
                                                 rhs, start=(i == 0), stop=(i == 3))
                        r_t = TP.tile([128, 512], f32, tag="rr", name="r_t", bufs=3)[:, :L]
                        nc.scalar.activation(r_t[:], pr[:, :L], AF.Sigmoid,
                                             bias=bur_sb[:, h2:h2 + 1])
                        nc.gpsimd.tensor_tensor(rm_t[h2][:], r_t[:], m_t[h2][:], op=OP.mult)
                for h2 in range(2):
                    nc.vector.tensor_tensor(s_nxt[h2][:], m_t[h2][:, :L // 2],
                                            m_t[h2][:, L // 2:], op=OP.add)
                    if d >= 2:
                        nc.vector.tensor_tensor(arm_nxt[h2][:], rm_t[h2][:, :L // 2],
                                                rm_t[h2][:, L // 2:], op=OP.add)
                continue
            for c0 in range(0, L // 2, S):
                m_pair, rm_pair = [], []
                for half in range(2):
                    cc = c0 + half * (L // 2)
                    m_t = [TP.tile([128, 512], f32r, tag="mh", name="m_t", bufs=4)[:, :S]
                           for k in range(2)]
                    for h2 in range(2):
                        pz = PS.tile([128, 512], f32, tag="ps", name="ps")
                        ph = PS.tile([128, 512], f32, tag="ps", name="ps")
                        for n0 in range(0, S, 512):
                            nw = min(512, S - n0)
                            argz = [(Wz_sb[0], X[0][:, cc + n0:cc + n0 + nw]),
                                    (Wz_sb[1], X[1][:, cc + n0:cc + n0 + nw]),
                                    (Wz_sb[2], s_cur[0][:, cc + n0:cc + n0 + nw]),
                                    (Wz_sb[3], s_cur[1][:, cc + n0:cc + n0 + nw])]
                            for i, (w, rhs) in enumerate(argz):
                                nc.tensor.matmul(pz[:, n0:n0 + nw],
                                                 w[:, 128 * h2:128 * (h2 + 1)], rhs,
                                                 start=(i == 0), stop=(i == 3))
                            argh = [(Wh_sb[0], X[0][:, cc + n0:cc + n0 + nw]),
                                    (Wh_sb[1], X[1][:, cc + n0:cc + n0 + nw]),
                                    (Wh_sb[2], arm_cur[0][:, cc + n0:cc + n0 + nw]),
                                    (Wh_sb[3], arm_cur[1][:, cc + n0:cc + n0 + nw])]
                            for i, (w, rhs) in enumerate(argh):
                                nc.tensor.matmul(ph[:, n0:n0 + nw],
                                                 w[:, 128 * h2:128 * (h2 + 1)], rhs,
                                                 start=(i == 0), stop=(i == 3))
                        z_t = TP.tile([128, 512], f32, tag="ew", name="z_t", bufs=4)[:, :S]
                        t_t = TP.tile([128, 512], f32, tag="ew", name="t_t", bufs=4)[:, :S]
                        nc.scalar.activation(z_t[:], pz[:, :S], AF.Sigmoid, bias=bz_sb[:, h2:h2 + 1])
                        nc.scalar.activation(t_t[:], ph[:, :S], AF.Tanh, bias=bh_sb[:, h2:h2 + 1])
                        dd = TP.tile([128, 512], f32, tag="ew", name="dd", bufs=4)[:, :S]
                        nc.vector.tensor_tensor(dd[:], t_t[:], s_cur[h2][:, cc:cc + S], op=OP.subtract)
                        ee = TP.tile([128, 512], f32, tag="ew", name="ee", bufs=4)[:, :S]
                        nc.vector.tensor_tensor(ee[:], z_t[:], dd[:], op=OP.mult)
                        nc.vector.tensor_tensor(m_t[h2][:], ee[:], s_cur[h2][:, cc:cc + S], op=OP.add)
                    m_pair.append(m_t)
                    if d >= 2:
                        rms = []
                        for h2 in range(2):
                            pr = PS.tile([128, 512], f32, tag="ps", name="ps")
                            for n0 in range(0, S, 512):
                                nw = min(512, S - n0)
                                xo = c0 + n0
                                argr = [(Wr_sb[0], Xp[0][:, xo:xo + nw]),
                                        (Wr_sb[1], Xp[1][:, xo:xo + nw]),
                                        (Ur_sb[0], m_t[0][:, n0:n0 + nw]),
                                        (Ur_sb[1], m_t[1][:, n0:n0 + nw])]
                                for i, (w, rhs) in enumerate(argr):
                                    nc.tensor.matmul(pr[:, n0:n0 + nw],
                                                     w[:, 128 * h2:128 * (h2 + 1)], rhs,
                                                     start=(i == 0), stop=(i == 3))
                            r_t = TP.tile([128, 512], f32, tag="rr", name="r_t", bufs=3)[:, :S]
                            nc.scalar.activation(r_t[:], pr[:, :S], AF.Sigmoid, bias=bur_sb[:, h2:h2 + 1])
                            rm_t = TP.tile([128, 512], f32, tag=f"rm{h2}", name="rm_t", bufs=2)[:, :S]
                            nc.gpsimd.tensor_tensor(rm_t[:], r_t[:], m_t[h2][:], op=OP.mult)
                            rms.append(rm_t)
                        rm_pair.append(rms)
                for h2 in range(2):
                    nc.vector.tensor_tensor(s_nxt[h2][:, c0:c0 + S],
                                            m_pair[0][h2][:], m_pair[1][h2][:], op=OP.add)
                    if d >= 2:
                        nc.vector.tensor_tensor(arm_nxt[h2][:, c0:c0 + S],
                                                rm_pair[0][h2][:], rm_pair[1][h2][:], op=OP.add)

        if phase == 3:
            for k in range(2):
                nc.sync.dma_start(dbg[128 * k:128 * (k + 1), 0:LW[4]], s_nxt[k][:].bitcast(f32))
                nc.sync.dma_start(dbg[128 * k:128 * (k + 1), 4096:4096 + LW[4]], arm_nxt[k][:].bitcast(f32))
        # ---- roots: h = relu(Wg^T [x_root; node_m] + bg) ----
        Xr = [xA[k][:, _OFF[0]:_OFF[0] + T] for k in range(2)]
        for h2 in (range(2) if phase >= 4 else ()):
            pg = PS.tile([128, 512], f32, tag="ps", name="ps")
            argg = [(Wg_sb[0], Xr[0]), (Wg_sb[1], Xr[1]),
                    (Wg_sb[2], s_nxt[0][:]), (Wg_sb[3], s_nxt[1][:])]
            for i, (w, rhs) in enumerate(argg):
                nc.tensor.matmul(pg[:, :T], w[:, 128 * h2:128 * (h2 + 1)], rhs,
                                 start=(i == 0), stop=(i == 3))
            o_t = SB.tile([128, T], f32, tag=f"o{h2}", name=f"o{h2}")
            nc.scalar.activation(o_t[:], pg[:, :T], AF.Relu, bias=bg_sb[:, h2:h2 + 1])
            nc.sync.dma_start(out[128 * h2:128 * (h2 + 1), :], o_t[:])

    nc.compile()
    return nc


def _in_maps(inputs):
    wid = np.asarray(inputs["wid"]).astype(np.int64)
    base = {k: np.ascontiguousarray(np.asarray(inputs[k], np.float32))
            for k in ("emb", "Wz", "bz", "Wr", "Ur", "bur", "Wh", "bh", "Wg", "bg")}
    base["ident"] = np.eye(128, dtype=np.float32)

    in_maps = []
    for c in range(NCORES):
        nn = (c * T + _NL_NODE[:, 0]) * N_TREE + _NL_NODE[:, 1]
        ln = (c * T + _LF_NODE[:, 0]) * N_TREE + _LF_NODE[:, 1]
        in_maps.append({**base, "widn": _wrap(wid[nn]), "widl": _wrap(wid[ln])})
    return in_maps


def kernel(**inputs):
    global _NC_CACHE
    from concourse import bass_utils

    in_maps = _in_maps(inputs)
    if _NC_CACHE is None:
        import os
        _NC_CACHE = _build(int(os.environ.get("K_PHASE", "4")))
    res = bass_utils.run_bass_kernel_spmd(_NC_CACHE, in_maps, core_ids=list(range(NCORES)))
    outs = [np.ascontiguousarray(np.asarray(r["out"]).T) for r in res.results]
    return np.concatenate(outs, 0).astype(np.float32)


if __name__ == "__main__":
    rng = np.random.default_rng(0)
    ins = {
        "wid": rng.integers(0, V, B * N_TREE).astype(np.int32),
        "emb": rng.standard_normal((V, H), dtype=np.float32),
        "Wz": rng.standard_normal((2 * H, H), dtype=np.float32) / 22.6,
        "bz": rng.standard_normal(H).astype(np.float32),
        "Wr": rng.standard_normal((H, H), dtype=np.float32) / 16.0,
        "Ur": rng.standard_normal((H, H), dtype=np.float32) / 16.0,
        "bur": rng.standard_normal(H).astype(np.float32),
        "Wh": rng.standard_normal((2 * H, H), dtype=np.float32) / 22.6,
        "bh": rng.standard_normal(H).astype(np.float32),
        "Wg": rng.standard_normal((2 * H, H), dtype=np.float32) / 22.6,
        "bg": rng.standard_normal(H).astype(np.float32),
    }
    o = kernel(**ins)
    print("kernel output", o.shape, o.dtype, float(np.abs(o).max()))

